# revision 1
# baseline (speedup 1.0000x reference)
"""AIMNet-style embedding kernel for 8 Trainium2 NeuronCores.

Data-parallel over the molecule batch B=8 (one molecule per core).
Host-side prep does layout transforms only (transpose ga/gr so the
contraction axis is on partitions, bf16 casts, small broadcast tables);
all FLOPs run on device.

Per-core device pipeline (molecule b):
  1. pair build:  X^T[128, 8128]  (127 DVE tensor_scalar ops, one per anchor atom i)
  2. combine MLP: C1^T = cw1^T @ X^T -> gelu -> G1^T ;  FP = G1^T chunks @ cw2
  3. grv:  afv^T @ grT slices  (per radial shift r)     -> Z^T k-tiles 0..15
  4. gav:  FP_k^T @ gaT k-tiles (64 accumulating steps) -> Z^T gav pieces
  5. embed MLP: ew1^T @ Z^T -> gelu -> A1^T ; ew2^T @ A1^T -> AEF^T -> out
"""

import numpy as np
import ml_dtypes

import concourse.bass as bass
import concourse.mybir as mybir
import concourse.tile as tile
from concourse import bacc
from concourse.bass_utils import run_bass_kernel_spmd

BF16NP = ml_dtypes.bfloat16
F32 = mybir.dt.float32
BF = mybir.dt.bfloat16

B, N, A = 8, 128, 64
Rr, Ra = 32, 16
P = N * (N - 1) // 2          # 8128
D = 32                        # d_pair
H, E = 512, 256
M2 = N * Ra                   # 2048 = gav output dim (n*Ra + r')
G2 = N * Rr                   # 4096 = grT cols (n*Rr + r)
NKT = (P + N - 1) // N        # 64 pair k-tiles (63 full + one of 64)
C_GRV = Rr * A                # 2048 embed in-features from grv

GELU = mybir.ActivationFunctionType.Gelu_apprx_tanh
IDENT = mybir.ActivationFunctionType.Identity
MULT = mybir.AluOpType.mult
ADD = mybir.AluOpType.add

_CACHE: dict = {}


def _build_nc():
    nc = bacc.Bacc("TRN2", target_bir_lowering=False)

    gaT = nc.dram_tensor("gaT", [P, M2], BF, kind="ExternalInput")
    grT = nc.dram_tensor("grT", [N, G2], BF, kind="ExternalInput")
    afv = nc.dram_tensor("afv", [N, A], BF, kind="ExternalInput")
    afv2 = nc.dram_tensor("afv2", [2 * A, N], F32, kind="ExternalInput")
    s1 = nc.dram_tensor("s1", [2 * A, N], F32, kind="ExternalInput")
    s2 = nc.dram_tensor("s2", [2 * A, N], F32, kind="ExternalInput")
    cw1 = nc.dram_tensor("cw1", [2 * A, 2 * A], F32, kind="ExternalInput")
    cw2 = nc.dram_tensor("cw2", [2 * A, D], BF, kind="ExternalInput")
    cb1 = nc.dram_tensor("cb1", [2 * A, 1], F32, kind="ExternalInput")
    cb2b = nc.dram_tensor("cb2b", [2 * A, D], F32, kind="ExternalInput")
    ew1 = nc.dram_tensor("ew1", [2560, H], BF, kind="ExternalInput")
    eb1 = nc.dram_tensor("eb1", [H], F32, kind="ExternalInput")
    ew2 = nc.dram_tensor("ew2", [H, E], BF, kind="ExternalInput")
    eb2 = nc.dram_tensor("eb2", [E], F32, kind="ExternalInput")
    out = nc.dram_tensor("out", [E, N], F32, kind="ExternalOutput")

    with tile.TileContext(nc) as tc:
        with (
            tc.tile_pool(name="const", bufs=1) as cp,
            tc.tile_pool(name="big", bufs=1) as bp,
            tc.tile_pool(name="ga", bufs=3) as gap,
        ):
            # ---- constants / small tensors ----
            cw1s = cp.tile([128, 128], F32)
            nc.sync.dma_start(out=cw1s, in_=cw1[:])
            cw2s = cp.tile([128, D], BF)
            nc.sync.dma_start(out=cw2s, in_=cw2[:])
            cb1s = cp.tile([128, 1], F32)
            nc.sync.dma_start(out=cb1s, in_=cb1[:])
            cb2bs = cp.tile([128, D], F32)
            nc.sync.dma_start(out=cb2bs, in_=cb2b[:])
            afvs = cp.tile([128, A], BF)
            nc.sync.dma_start(out=afvs, in_=afv[:])
            afv2s = cp.tile([128, N], F32)
            nc.sync.dma_start(out=afv2s, in_=afv2[:])
            s1s = cp.tile([128, N], F32)
            nc.sync.dma_start(out=s1s, in_=s1[:])
            s2s = cp.tile([128, N], F32)
            nc.sync.dma_start(out=s2s, in_=s2[:])
            eb1s = cp.tile([128, 4], F32)
            nc.sync.dma_start(out=eb1s, in_=eb1[:].rearrange("(c p) -> p c", p=128))
            eb2s = cp.tile([128, 2], F32)
            nc.sync.dma_start(out=eb2s, in_=eb2[:].rearrange("(c p) -> p c", p=128))
            # embed weights: grv rows as [128, 16, 512], gav rows as [32, 16, 512]
            ew1s = cp.tile([128, 16, H], BF)
            nc.sync.dma_start(
                out=ew1s, in_=ew1[0:C_GRV, :].rearrange("(t p) h -> p t h", p=128)
            )
            ew1gs = cp.tile([32, 16, H], BF)
            nc.sync.dma_start(
                out=ew1gs, in_=ew1[C_GRV:2560, :].rearrange("(r d) h -> d r h", d=32)
            )
            ew2s = cp.tile([128, 4, E], BF)
            nc.sync.dma_start(
                out=ew2s, in_=ew2[:].rearrange("(t p) e -> p t e", p=128)
            )
            grts = cp.tile([128, G2], BF)
            nc.sync.dma_start(out=grts, in_=grT[:])

            # ---- persistent intermediates ----
            xt = bp.tile([128, P], F32)          # X^T  (pair features)
            g1t = bp.tile([128, P], BF)          # gelu(C1)^T
            fps = bp.tile([128, NKT * D], BF)    # FP, k-tile q at cols [q*32, q*32+32)
            zt = bp.tile([128, 16 * N], BF)      # Z^T grv part, k-tile kt at cols kt*128
            ztg = bp.tile([32, 16 * N], BF)      # Z^T gav part, piece r' at cols r'*128
            a1t = bp.tile([128, 4 * N], BF)      # A1^T, h-chunk hc at cols hc*128
            aeft = bp.tile([128, 2, N], F32)     # AEF^T chunks

            # ---- stage 1: pair features X^T ----
            # block i (i=0..126): pairs (i, j) j=i+1..127, width w=127-i
            off = 0
            for i in range(N - 1):
                w = N - 1 - i
                nc.vector.tensor_scalar(
                    out=xt[:, off:off + w],
                    in0=afv2s[:, i + 1:N],
                    scalar1=s1s[:, i:i + 1],
                    scalar2=s2s[:, i:i + 1],
                    op0=MULT,
                    op1=ADD,
                )
                off += w
            assert off == P

            grt_r = grts[:].rearrange("p (n r) -> p r n", r=Rr)

            with (
                tc.tile_pool(name="psA", bufs=2, space="PSUM") as psA,
                tc.tile_pool(name="psGrv", bufs=1, space="PSUM") as psGrv,
            ):
                # ---- stage 3: grv ----
                # psum piece r: [64(a), 128(n)] at partition base (r%2)*64, col kt=r//2
                ps_grv = psGrv.tile([128, 16, N], F32)
                for r in range(Rr):
                    base = (r % 2) * 64
                    nc.tensor.matmul(
                        ps_grv[base:base + 64, r // 2, :],
                        afvs[:, :],
                        grt_r[:, r, :],
                        start=True,
                        stop=True,
                        tile_position=(0, base),
                    )
                for kt in range(16):
                    nc.vector.tensor_copy(zt[:, kt * N:(kt + 1) * N], ps_grv[:, kt, :])

                # ---- stage 2: combine MLP ----
                for pc in range(16):
                    w = min(512, P - pc * 512)
                    ps = psA.tile([128, 512], F32, tag="c1")
                    nc.tensor.matmul(
                        ps[:, 0:w], cw1s[:, :], xt[:, pc * 512:pc * 512 + w],
                        start=True, stop=True,
                    )
                    nc.scalar.activation(
                        g1t[:, pc * 512:pc * 512 + w], ps[:, 0:w], GELU,
                        bias=cb1s[:, 0:1], scale=1.0,
                    )
                for q in range(NKT):
                    kw = min(128, P - q * 128)
                    ps = psA.tile([128, D], F32, tag="fp")
                    nc.tensor.matmul(
                        ps[0:kw, :], g1t[:, q * 128:q * 128 + kw], cw2s[:, :],
                        start=True, stop=True,
                    )
                    nc.vector.tensor_tensor(
                        out=fps[0:kw, q * D:(q + 1) * D],
                        in0=ps[0:kw, :],
                        in1=cb2bs[0:kw, :],
                        op=ADD,
                    )

            # ---- stage 4: gav (the big stream) ----
            with tc.tile_pool(name="psGav", bufs=1, space="PSUM") as psGav:
                psg = psGav.tile([32, M2], F32)
                for dm in range(32):
                    ga_t = gap.tile([128, 2, M2], BF, tag="ga")
                    if dm < 31:
                        nc.sync.dma_start(
                            out=ga_t,
                            in_=gaT[dm * 256:(dm + 1) * 256, :].rearrange(
                                "(two p) m -> p two m", two=2
                            ),
                        )
                    else:
                        nc.sync.dma_start(
                            out=ga_t[:, 0, :], in_=gaT[7936:8064, :]
                        )
                        nc.sync.dma_start(
                            out=ga_t[0:64, 1, :], in_=gaT[8064:8128, :]
                        )
                    for half in range(2):
                        kt = dm * 2 + half
                        kw = 64 if kt == NKT - 1 else 128
                        for mc in range(4):
                            nc.tensor.matmul(
                                psg[:, mc * 512:(mc + 1) * 512],
                                fps[0:kw, kt * D:(kt + 1) * D],
                                ga_t[0:kw, half, mc * 512:(mc + 1) * 512],
                                start=(kt == 0),
                                stop=(kt == NKT - 1),
                            )
                psg_r = psg[:].rearrange("d (n r) -> d r n", r=Ra)
                for rp in range(Ra):
                    nc.vector.tensor_copy(ztg[:, rp * N:(rp + 1) * N], psg_r[:, rp, :])

            # ---- stage 5: embedding MLP ----
            with tc.tile_pool(name="psE", bufs=2, space="PSUM") as psE:
                for hc in range(4):
                    ps1 = psE.tile([128, N], F32, tag="a1")
                    for kt in range(16):
                        nc.tensor.matmul(
                            ps1,
                            ew1s[:, kt, hc * 128:(hc + 1) * 128],
                            zt[:, kt * N:(kt + 1) * N],
                            start=(kt == 0),
                            stop=False,
                        )
                    for rp in range(Ra):
                        nc.tensor.matmul(
                            ps1,
                            ew1gs[:, rp, hc * 128:(hc + 1) * 128],
                            ztg[:, rp * N:(rp + 1) * N],
                            start=False,
                            stop=(rp == Ra - 1),
                        )
                    nc.scalar.activation(
                        a1t[:, hc * N:(hc + 1) * N], ps1, GELU,
                        bias=eb1s[:, hc:hc + 1], scale=1.0,
                    )
                for ec in range(2):
                    ps2 = psE.tile([128, N], F32, tag="aef")
                    for ht in range(4):
                        nc.tensor.matmul(
                            ps2,
                            ew2s[:, ht, ec * 128:(ec + 1) * 128],
                            a1t[:, ht * N:(ht + 1) * N],
                            start=(ht == 0),
                            stop=(ht == 3),
                        )
                    nc.scalar.activation(
                        aeft[:, ec, :], ps2, IDENT, bias=eb2s[:, ec:ec + 1], scale=1.0,
                    )

                nc.sync.dma_start(
                    out=out[:].rearrange("(c e) n -> e c n", c=2), in_=aeft
                )

    nc.compile()
    return nc


def _get_nc():
    if "nc" not in _CACHE:
        _CACHE["nc"] = _build_nc()
    return _CACHE["nc"]


def _prep_in_maps(gr, ga, afv, cw1, cb1, cw2, cb2, ew1, eb1, ew2, eb2):
    gr = np.asarray(gr, np.float32)
    ga = np.asarray(ga, np.float32)
    afv = np.asarray(afv, np.float32)
    cw1 = np.asarray(cw1, np.float32)
    cb1 = np.asarray(cb1, np.float32)
    cw2 = np.asarray(cw2, np.float32)
    cb2 = np.asarray(cb2, np.float32)
    ew1 = np.asarray(ew1, np.float32)
    eb1 = np.asarray(eb1, np.float32)
    ew2 = np.asarray(ew2, np.float32)
    eb2 = np.asarray(eb2, np.float32)

    shared = {
        "cw1": np.ascontiguousarray(cw1),
        "cw2": np.ascontiguousarray(cw2.astype(BF16NP)),
        "cb1": np.ascontiguousarray(cb1.reshape(2 * A, 1)),
        "cb2b": np.ascontiguousarray(np.broadcast_to(cb2, (2 * A, D))),
        "ew1": np.ascontiguousarray(ew1.astype(BF16NP)),
        "eb1": np.ascontiguousarray(eb1),
        "ew2": np.ascontiguousarray(ew2.astype(BF16NP)),
        "eb2": np.ascontiguousarray(eb2),
    }
    in_maps = []
    ones64 = np.ones((A, N), np.float32)
    zeros64 = np.zeros((A, N), np.float32)
    for b in range(B):
        afvT = np.ascontiguousarray(afv[b].T)  # [64, 128]
        m = dict(shared)
        m["gaT"] = np.ascontiguousarray(
            ga[b].reshape(M2, P).T.astype(BF16NP)
        )
        m["grT"] = np.ascontiguousarray(
            gr[b].reshape(G2, N).T.astype(BF16NP)
        )
        m["afv"] = np.ascontiguousarray(afv[b].astype(BF16NP))
        m["afv2"] = np.ascontiguousarray(np.concatenate([afvT, afvT], axis=0))
        m["s1"] = np.ascontiguousarray(np.concatenate([ones64, afvT], axis=0))
        m["s2"] = np.ascontiguousarray(np.concatenate([afvT, zeros64], axis=0))
        in_maps.append(m)
    return in_maps


def run(inputs: dict, trace: bool = False):
    """Returns ((aef, afv), exec_time_ns_or_None)."""
    nc = _get_nc()
    in_maps = _prep_in_maps(**inputs)
    res = run_bass_kernel_spmd(nc, in_maps, core_ids=list(range(B)), trace=trace)
    aef = np.stack(
        [np.ascontiguousarray(res.results[b]["out"].T) for b in range(B)], axis=0
    )
    afv = np.asarray(inputs["afv"], np.float32)
    return (aef, afv), res.exec_time_ns


def kernel(**inputs) -> np.ndarray:
    (aef, afv), _ = run(inputs, trace=False)
    return aef, afv


# revision 3
# speedup vs baseline: 1.2097x; 1.2097x over previous
"""AIMNet-style embedding kernel for 8 Trainium2 NeuronCores.

Data-parallel over the molecule batch B=8 (one molecule per core).
Host-side prep does layout transforms only (transpose ga/gr so the
contraction axis is on partitions, bf16 casts, small broadcast tables);
all FLOPs run on device.

Per-core device pipeline (molecule b):
  1. pair build:  X^T[128, 8128]  (one tensor_scalar per anchor atom i,
     split across DVE and GpSimd)
  2. combine MLP: C1^T = cw1^T @ X^T -> gelu -> G1^T ;  FP = G1^T chunks @ cw2
  3. grv:  afv^T @ grT slices  (per radial shift r)     -> Z^T k-tiles 0..15
  4. gav:  FP_k^T @ gaT k-tiles (64 accumulating steps) -> Z^T gav pieces
  5. embed MLP: accumulate psum[n, 512] over Z^T k-tiles (Z as stationary,
     ew1 as moving; eb1 folded in as a rank-1 matmul), gelu, PE-transpose
     A1 -> A1^T, then ew2^T @ A1^T -> AEF^T (+eb2) -> out
"""

import numpy as np
import ml_dtypes

import concourse.bass as bass
import concourse.mybir as mybir
import concourse.tile as tile
from concourse import bacc
from concourse.bass_utils import run_bass_kernel_spmd
from concourse.masks import make_identity

BF16NP = ml_dtypes.bfloat16
F32 = mybir.dt.float32
BF = mybir.dt.bfloat16

B, N, A = 8, 128, 64
Rr, Ra = 32, 16
P = N * (N - 1) // 2          # 8128
D = 32                        # d_pair
H, E = 512, 256
M2 = N * Ra                   # 2048 = gav output dim (n*Ra + r')
G2 = N * Rr                   # 4096 = grT cols (n*Rr + r)
NKT = (P + N - 1) // N        # 64 pair k-tiles (63 full + one of 64)
C_GRV = Rr * A                # 2048 embed in-features from grv

GELU = mybir.ActivationFunctionType.Gelu_apprx_tanh
IDENT = mybir.ActivationFunctionType.Identity
MULT = mybir.AluOpType.mult
ADD = mybir.AluOpType.add

_CACHE: dict = {}


def _build_nc():
    nc = bacc.Bacc("TRN2", target_bir_lowering=False)

    gaT = nc.dram_tensor("gaT", [P, M2], BF, kind="ExternalInput")
    grT = nc.dram_tensor("grT", [N, G2], BF, kind="ExternalInput")
    afv = nc.dram_tensor("afv", [N, A], BF, kind="ExternalInput")
    afv2 = nc.dram_tensor("afv2", [2 * A, N], F32, kind="ExternalInput")
    s1 = nc.dram_tensor("s1", [2 * A, N], F32, kind="ExternalInput")
    s2 = nc.dram_tensor("s2", [2 * A, N], F32, kind="ExternalInput")
    cw1 = nc.dram_tensor("cw1", [2 * A, 2 * A], BF, kind="ExternalInput")
    cw2 = nc.dram_tensor("cw2", [2 * A, D], BF, kind="ExternalInput")
    cb1 = nc.dram_tensor("cb1", [2 * A, 1], F32, kind="ExternalInput")
    cb2b = nc.dram_tensor("cb2b", [2 * A, D], F32, kind="ExternalInput")
    ew1 = nc.dram_tensor("ew1", [2560, H], BF, kind="ExternalInput")
    eb1r = nc.dram_tensor("eb1r", [1, H], BF, kind="ExternalInput")
    ew2 = nc.dram_tensor("ew2", [H, E], BF, kind="ExternalInput")
    eb2 = nc.dram_tensor("eb2", [E], F32, kind="ExternalInput")
    out = nc.dram_tensor("out", [E, N], F32, kind="ExternalOutput")

    with tile.TileContext(nc) as tc:
        with (
            tc.tile_pool(name="const", bufs=1) as cp,
            tc.tile_pool(name="big", bufs=1) as bp,
            tc.tile_pool(name="ga", bufs=8) as gap,
        ):
            # ---- constants / small tensors ----
            cw1s = cp.tile([128, 128], BF)
            nc.sync.dma_start(out=cw1s, in_=cw1[:])
            cw2s = cp.tile([128, D], BF)
            nc.sync.dma_start(out=cw2s, in_=cw2[:])
            cb1s = cp.tile([128, 1], F32)
            nc.sync.dma_start(out=cb1s, in_=cb1[:])
            cb2bs = cp.tile([128, D], F32)
            nc.sync.dma_start(out=cb2bs, in_=cb2b[:])
            afvs = cp.tile([128, A], BF)
            nc.sync.dma_start(out=afvs, in_=afv[:])
            afv2s = cp.tile([128, N], F32)
            nc.sync.dma_start(out=afv2s, in_=afv2[:])
            s1s = cp.tile([128, N], F32)
            nc.sync.dma_start(out=s1s, in_=s1[:])
            s2s = cp.tile([128, N], F32)
            nc.sync.dma_start(out=s2s, in_=s2[:])
            eb1rs = cp.tile([1, H], BF)
            nc.sync.dma_start(out=eb1rs, in_=eb1r[:])
            eb2s = cp.tile([128, 2], F32)
            nc.sync.dma_start(out=eb2s, in_=eb2[:].rearrange("(c p) -> p c", p=128))
            # embed weights: grv rows as [128, 16, 512], gav rows as [32, 16, 512]
            ew1s = cp.tile([128, 16, H], BF)
            nc.sync.dma_start(
                out=ew1s, in_=ew1[0:C_GRV, :].rearrange("(t p) h -> p t h", p=128)
            )
            ew1gs = cp.tile([32, 16, H], BF)
            nc.sync.dma_start(
                out=ew1gs, in_=ew1[C_GRV:2560, :].rearrange("(r d) h -> d r h", d=32)
            )
            ew2s = cp.tile([128, 4, E], BF)
            nc.sync.dma_start(
                out=ew2s, in_=ew2[:].rearrange("(t p) e -> p t e", p=128)
            )
            grts = cp.tile([128, G2], BF)
            nc.sync.dma_start(out=grts, in_=grT[:])

            ones1 = cp.tile([1, N], BF)
            nc.vector.memset(ones1, 1.0)
            ident = cp.tile([128, 128], BF)
            make_identity(nc, ident)

            # ---- persistent intermediates ----
            xt = bp.tile([128, P], BF)           # X^T  (pair features)
            g1t = bp.tile([128, P], BF)          # gelu(C1)^T
            fps = bp.tile([128, NKT * D], BF)    # FP, k-tile q at cols [q*32, q*32+32)
            zt = bp.tile([128, 16 * N], BF)      # Z^T grv part, k-tile kt at cols kt*128
            ztg = bp.tile([32, 16 * N], BF)      # Z^T gav part, piece r' at cols r'*128
            a1 = bp.tile([128, H], BF)           # A1 [n, h]
            a1t = bp.tile([128, 4, N], BF)       # A1^T, h-chunk ht at [:, ht, :]
            aeft = bp.tile([128, 2, N], F32)     # AEF^T chunks

            # ---- stage 1: pair features X^T ----
            # block i (i=0..126): pairs (i, j) j=i+1..127, width w=127-i
            # wide blocks on DVE, narrow tail on GpSimd (runs in parallel)
            off = 0
            for i in range(N - 1):
                w = N - 1 - i
                eng = nc.vector if i < 72 else nc.gpsimd
                eng.tensor_scalar(
                    out=xt[:, off:off + w],
                    in0=afv2s[:, i + 1:N],
                    scalar1=s1s[:, i:i + 1],
                    scalar2=s2s[:, i:i + 1],
                    op0=MULT,
                    op1=ADD,
                )
                off += w
            assert off == P

            grt_r = grts[:].rearrange("p (n r) -> p r n", r=Rr)

            with (
                tc.tile_pool(name="psA", bufs=2, space="PSUM") as psA,
                tc.tile_pool(name="psGrv", bufs=1, space="PSUM") as psGrv,
            ):
                # ---- stage 3: grv ----
                # psum piece r: [64(a), 128(n)] at partition base (r%2)*64, col kt=r//2
                ps_grv = psGrv.tile([128, 16, N], F32)
                for r in range(Rr):
                    base = (r % 2) * 64
                    nc.tensor.matmul(
                        ps_grv[base:base + 64, r // 2, :],
                        afvs[:, :],
                        grt_r[:, r, :],
                        start=True,
                        stop=True,
                        tile_position=(0, base),
                    )
                for kt in range(16):
                    nc.vector.tensor_copy(zt[:, kt * N:(kt + 1) * N], ps_grv[:, kt, :])

                # ---- stage 2: combine MLP ----
                for pc in range(16):
                    w = min(512, P - pc * 512)
                    ps = psA.tile([128, 512], F32, tag="c1")
                    nc.tensor.matmul(
                        ps[:, 0:w], cw1s[:, :], xt[:, pc * 512:pc * 512 + w],
                        start=True, stop=True,
                    )
                    nc.scalar.activation(
                        g1t[:, pc * 512:pc * 512 + w], ps[:, 0:w], GELU,
                        bias=cb1s[:, 0:1], scale=1.0,
                    )
                for q in range(NKT):
                    kw = min(128, P - q * 128)
                    ps = psA.tile([128, D], F32, tag="fp")
                    nc.tensor.matmul(
                        ps[0:kw, :], g1t[:, q * 128:q * 128 + kw], cw2s[:, :],
                        start=True, stop=True,
                    )
                    nc.vector.tensor_tensor(
                        out=fps[0:kw, q * D:(q + 1) * D],
                        in0=ps[0:kw, :],
                        in1=cb2bs[0:kw, :],
                        op=ADD,
                    )

            # ---- stage 4: gav (the big stream) ----
            with tc.tile_pool(name="psGav", bufs=1, space="PSUM") as psGav:
                psg = psGav.tile([32, M2], F32)
                for dm in range(32):
                    ga_t = gap.tile([128, 2, M2], BF, tag="ga")
                    if dm < 31:
                        nc.sync.dma_start(
                            out=ga_t,
                            in_=gaT[dm * 256:(dm + 1) * 256, :].rearrange(
                                "(two p) m -> p two m", two=2
                            ),
                        )
                    else:
                        nc.sync.dma_start(
                            out=ga_t[:, 0, :], in_=gaT[7936:8064, :]
                        )
                        nc.sync.dma_start(
                            out=ga_t[0:64, 1, :], in_=gaT[8064:8128, :]
                        )
                    for half in range(2):
                        kt = dm * 2 + half
                        kw = 64 if kt == NKT - 1 else 128
                        for mc in range(4):
                            nc.tensor.matmul(
                                psg[:, mc * 512:(mc + 1) * 512],
                                fps[0:kw, kt * D:(kt + 1) * D],
                                ga_t[0:kw, half, mc * 512:(mc + 1) * 512],
                                start=(kt == 0),
                                stop=(kt == NKT - 1),
                            )
                psg_r = psg[:].rearrange("d (n r) -> d r n", r=Ra)
                for rp in range(Ra):
                    nc.vector.tensor_copy(ztg[:, rp * N:(rp + 1) * N], psg_r[:, rp, :])

            # ---- stage 5: embedding MLP ----
            with tc.tile_pool(name="psE", bufs=2, space="PSUM") as psE:
                # A1[n, h] accumulation: rank-1 eb1 + 16 grv k-tiles (ready
                # early) + 16 gav pieces (the only tail after the big stream)
                ps1 = psE.tile([128, H], F32, tag="a1")
                nc.tensor.matmul(ps1, ones1, eb1rs, start=True, stop=False)
                for kt in range(16):
                    nc.tensor.matmul(
                        ps1,
                        zt[:, kt * N:(kt + 1) * N],
                        ew1s[:, kt, :],
                        start=False,
                        stop=False,
                    )
                for rp in range(Ra):
                    nc.tensor.matmul(
                        ps1,
                        ztg[0:32, rp * N:(rp + 1) * N],
                        ew1gs[0:32, rp, :],
                        start=False,
                        stop=(rp == Ra - 1),
                    )
                nc.scalar.activation(a1, ps1, GELU, bias=0.0, scale=1.0)
                # transpose A1 -> A1^T via PE
                for ht in range(4):
                    tr = psE.tile([128, N], BF, tag="tr")
                    nc.tensor.transpose(tr, a1[:, ht * 128:(ht + 1) * 128], ident)
                    nc.vector.tensor_copy(a1t[:, ht, :], tr)
                for ec in range(2):
                    ps2 = psE.tile([128, N], F32, tag="aef")
                    for ht in range(4):
                        nc.tensor.matmul(
                            ps2,
                            ew2s[:, ht, ec * 128:(ec + 1) * 128],
                            a1t[:, ht, :],
                            start=(ht == 0),
                            stop=(ht == 3),
                        )
                    nc.scalar.activation(
                        aeft[:, ec, :], ps2, IDENT, bias=eb2s[:, ec:ec + 1], scale=1.0,
                    )

                nc.sync.dma_start(
                    out=out[:].rearrange("(c e) n -> e c n", c=2), in_=aeft
                )

    nc.compile()
    return nc


def _get_nc():
    if "nc" not in _CACHE:
        _CACHE["nc"] = _build_nc()
    return _CACHE["nc"]


def _prep_in_maps(gr, ga, afv, cw1, cb1, cw2, cb2, ew1, eb1, ew2, eb2):
    gr = np.asarray(gr, np.float32)
    ga = np.asarray(ga, np.float32)
    afv = np.asarray(afv, np.float32)
    cw1 = np.asarray(cw1, np.float32)
    cb1 = np.asarray(cb1, np.float32)
    cw2 = np.asarray(cw2, np.float32)
    cb2 = np.asarray(cb2, np.float32)
    ew1 = np.asarray(ew1, np.float32)
    eb1 = np.asarray(eb1, np.float32)
    ew2 = np.asarray(ew2, np.float32)
    eb2 = np.asarray(eb2, np.float32)

    shared = {
        "cw1": np.ascontiguousarray(cw1.astype(BF16NP)),
        "cw2": np.ascontiguousarray(cw2.astype(BF16NP)),
        "cb1": np.ascontiguousarray(cb1.reshape(2 * A, 1)),
        "cb2b": np.ascontiguousarray(np.broadcast_to(cb2, (2 * A, D))),
        "ew1": np.ascontiguousarray(ew1.astype(BF16NP)),
        "eb1r": np.ascontiguousarray(eb1.reshape(1, H).astype(BF16NP)),
        "ew2": np.ascontiguousarray(ew2.astype(BF16NP)),
        "eb2": np.ascontiguousarray(eb2),
    }
    in_maps = []
    ones64 = np.ones((A, N), np.float32)
    zeros64 = np.zeros((A, N), np.float32)
    for b in range(B):
        afvT = np.ascontiguousarray(afv[b].T)  # [64, 128]
        m = dict(shared)
        m["gaT"] = np.ascontiguousarray(
            ga[b].reshape(M2, P).T.astype(BF16NP)
        )
        m["grT"] = np.ascontiguousarray(
            gr[b].reshape(G2, N).T.astype(BF16NP)
        )
        m["afv"] = np.ascontiguousarray(afv[b].astype(BF16NP))
        m["afv2"] = np.ascontiguousarray(np.concatenate([afvT, afvT], axis=0))
        m["s1"] = np.ascontiguousarray(np.concatenate([ones64, afvT], axis=0))
        m["s2"] = np.ascontiguousarray(np.concatenate([afvT, zeros64], axis=0))
        in_maps.append(m)
    return in_maps


def run(inputs: dict, trace: bool = False):
    """Returns ((aef, afv), exec_time_ns_or_None)."""
    nc = _get_nc()
    in_maps = _prep_in_maps(**inputs)
    res = run_bass_kernel_spmd(nc, in_maps, core_ids=list(range(B)), trace=trace)
    aef = np.stack(
        [np.ascontiguousarray(res.results[b]["out"].T) for b in range(B)], axis=0
    )
    afv = np.asarray(inputs["afv"], np.float32)
    return (aef, afv), res.exec_time_ns


def kernel(**inputs) -> np.ndarray:
    (aef, afv), _ = run(inputs, trace=False)
    return aef, afv


# revision 10
# speedup vs baseline: 1.2837x; 1.0611x over previous
"""AIMNet-style embedding kernel for 8 Trainium2 NeuronCores.

Data-parallel over the molecule batch B=8 (one molecule per core).
Host-side prep does layout transforms only (transpose ga/gr so the
contraction axis is on partitions, bf16 casts, small broadcast tables);
all FLOPs run on device.

Per-core device pipeline (molecule b):
  1. pair build:  X^T[128, 8128]  (one tensor_scalar per anchor atom i,
     split across DVE and GpSimd)
  2. combine MLP: C1^T = cw1^T @ X^T -> gelu -> G1^T ;  FP = G1^T chunks @ cw2
  3. grv:  afv^T @ grT slices  (per radial shift r)     -> Z^T k-tiles 0..15
  4. gav:  FP_k^T @ gaT k-tiles (64 accumulating steps) -> Z^T gav pieces
  5. embed MLP: accumulate psum[n, 512] over Z^T k-tiles (Z as stationary,
     ew1 as moving; eb1 folded in as a rank-1 matmul), gelu, PE-transpose
     A1 -> A1^T, then ew2^T @ A1^T -> AEF^T (+eb2) -> out
"""

import numpy as np
import ml_dtypes

import concourse.bass as bass
import concourse.mybir as mybir
import concourse.tile as tile
from concourse import bacc
from concourse.bass_utils import run_bass_kernel_spmd
from concourse.masks import make_identity

BF16NP = ml_dtypes.bfloat16
F32 = mybir.dt.float32
BF = mybir.dt.bfloat16

B, N, A = 8, 128, 64
Rr, Ra = 32, 16
P = N * (N - 1) // 2          # 8128
D = 32                        # d_pair
H, E = 512, 256
M2 = N * Ra                   # 2048 = gav output dim (n*Ra + r')
G2 = N * Rr                   # 4096 = grT cols (n*Rr + r)
NKT = (P + N - 1) // N        # 64 pair k-tiles (63 full + one of 64)
C_GRV = Rr * A                # 2048 embed in-features from grv

GELU = mybir.ActivationFunctionType.Gelu_apprx_tanh
IDENT = mybir.ActivationFunctionType.Identity
MULT = mybir.AluOpType.mult
ADD = mybir.AluOpType.add

_CACHE: dict = {}


def _build_nc():
    nc = bacc.Bacc("TRN2", target_bir_lowering=False)

    gaT = nc.dram_tensor("gaT", [P, M2], BF, kind="ExternalInput")
    grT = nc.dram_tensor("grT", [N, G2], BF, kind="ExternalInput")
    afv = nc.dram_tensor("afv", [N, A], BF, kind="ExternalInput")
    afv2 = nc.dram_tensor("afv2", [2 * A, N], F32, kind="ExternalInput")
    s1 = nc.dram_tensor("s1", [2 * A, N], F32, kind="ExternalInput")
    s2 = nc.dram_tensor("s2", [2 * A, N], F32, kind="ExternalInput")
    cw1 = nc.dram_tensor("cw1", [2 * A, 2 * A], BF, kind="ExternalInput")
    cw2 = nc.dram_tensor("cw2", [2 * A, D], BF, kind="ExternalInput")
    cb1 = nc.dram_tensor("cb1", [2 * A, 1], F32, kind="ExternalInput")
    cb2b = nc.dram_tensor("cb2b", [2 * A, D], F32, kind="ExternalInput")
    ew1 = nc.dram_tensor("ew1", [2560, H], BF, kind="ExternalInput")
    eb1r = nc.dram_tensor("eb1r", [1, H], BF, kind="ExternalInput")
    ew2 = nc.dram_tensor("ew2", [H, E], BF, kind="ExternalInput")
    eb2 = nc.dram_tensor("eb2", [E], F32, kind="ExternalInput")
    out = nc.dram_tensor("out", [E, N], F32, kind="ExternalOutput")

    with tile.TileContext(nc) as tc:
        with (
            tc.tile_pool(name="const", bufs=1) as cp,
            tc.tile_pool(name="big", bufs=1) as bp,
            tc.tile_pool(name="ga", bufs=10) as gap,
        ):
            # ---- constants / small tensors ----
            cw1s = cp.tile([128, 128], BF)
            nc.sync.dma_start(out=cw1s, in_=cw1[:])
            cw2s = cp.tile([128, D], BF)
            nc.sync.dma_start(out=cw2s, in_=cw2[:])
            cb1s = cp.tile([128, 1], F32)
            nc.sync.dma_start(out=cb1s, in_=cb1[:])
            cb2bs = cp.tile([128, D], F32)
            nc.sync.dma_start(out=cb2bs, in_=cb2b[:])
            afvs = cp.tile([128, A], BF)
            nc.sync.dma_start(out=afvs, in_=afv[:])
            afv2s = cp.tile([128, N], F32)
            nc.sync.dma_start(out=afv2s, in_=afv2[:])
            s1s = cp.tile([128, N], F32)
            nc.sync.dma_start(out=s1s, in_=s1[:])
            s2s = cp.tile([128, N], F32)
            nc.sync.dma_start(out=s2s, in_=s2[:])
            eb1rs = cp.tile([1, H], BF)
            nc.sync.dma_start(out=eb1rs, in_=eb1r[:])
            eb2s = cp.tile([128, 2], F32)
            nc.sync.dma_start(out=eb2s, in_=eb2[:].rearrange("(c p) -> p c", p=128))
            # embed weights: grv rows as [128, 16, 512], gav rows as [32, 16, 512]
            ew1s = cp.tile([128, 16, H], BF)
            nc.sync.dma_start(
                out=ew1s, in_=ew1[0:C_GRV, :].rearrange("(t p) h -> p t h", p=128)
            )
            ew1gs = cp.tile([32, 16, H], BF)
            nc.sync.dma_start(
                out=ew1gs, in_=ew1[C_GRV:2560, :].rearrange("(r d) h -> d r h", d=32)
            )
            ew2s = cp.tile([128, 4, E], BF)
            nc.sync.dma_start(
                out=ew2s, in_=ew2[:].rearrange("(t p) e -> p t e", p=128)
            )
            grts = cp.tile([128, G2], BF)
            nc.sync.dma_start(out=grts, in_=grT[:])

            ones1 = cp.tile([1, N], BF)
            nc.vector.memset(ones1, 1.0)
            ident = cp.tile([128, 128], BF)
            make_identity(nc, ident)

            # ---- persistent intermediates ----
            xt = bp.tile([128, P], BF)           # X^T  (pair features)
            g1t = bp.tile([128, P], BF)          # gelu(C1)^T
            fps = bp.tile([128, NKT * D], BF)    # FP, k-tile q at cols [q*32, q*32+32)
            zt = bp.tile([128, 16 * N], BF)      # Z^T grv part, k-tile kt at cols kt*128
            ztg = bp.tile([32, 16 * N], BF)      # Z^T gav part, piece r' at cols r'*128
            a1 = bp.tile([128, H], BF)           # A1 [n, h]
            a1t = bp.tile([128, 4, N], BF)       # A1^T, h-chunk ht at [:, ht, :]
            aeft = bp.tile([128, 2, N], F32)     # AEF^T chunks

            # ---- stage 1: pair features X^T ----
            # block i (i=0..126): pairs (i, j) j=i+1..127, width w=127-i
            # wide blocks on DVE, narrow tail on GpSimd (runs in parallel)
            off = 0
            for i in range(N - 1):
                w = N - 1 - i
                eng = nc.gpsimd if i % 3 == 2 else nc.vector
                eng.tensor_scalar(
                    out=xt[:, off:off + w],
                    in0=afv2s[:, i + 1:N],
                    scalar1=s1s[:, i:i + 1],
                    scalar2=s2s[:, i:i + 1],
                    op0=MULT,
                    op1=ADD,
                )
                off += w
            assert off == P

            with (
                tc.tile_pool(name="psA", bufs=2, space="PSUM") as psA,
                tc.tile_pool(name="psGrv", bufs=1, space="PSUM") as psGrv,
            ):
                # ---- stage 2: combine MLP ----
                for pc in range(16):
                    w = min(512, P - pc * 512)
                    ps = psA.tile([128, 512], F32, tag="c1")
                    nc.tensor.matmul(
                        ps[:, 0:w], cw1s[:, :], xt[:, pc * 512:pc * 512 + w],
                        start=True, stop=True,
                    )
                    nc.scalar.activation(
                        g1t[:, pc * 512:pc * 512 + w], ps[:, 0:w], GELU,
                        bias=cb1s[:, 0:1], scale=1.0,
                    )
                for q in range(NKT):
                    kw = min(128, P - q * 128)
                    ps = psA.tile([128, D], F32, tag="fp")
                    nc.tensor.matmul(
                        ps[0:kw, :], g1t[:, q * 128:q * 128 + kw], cw2s[:, :],
                        start=True, stop=True,
                    )
                    nc.vector.tensor_tensor(
                        out=fps[0:kw, q * D:(q + 1) * D],
                        in0=ps[0:kw, :],
                        in1=cb2bs[0:kw, :],
                        op=ADD,
                    )

                # ---- stage 3: grv (emitted after combine so Tile biases
                # PE toward the gav-feeding chain; grT is r-major so the
                # moving operand is contiguous) ----
                # psum piece r: [64(a), 128(n)] at partition base (r%2)*64
                ps_grv = psGrv.tile([128, 16, N], F32)
                for r in range(Rr):
                    base = (r % 2) * 64
                    nc.tensor.matmul(
                        ps_grv[base:base + 64, r // 2, :],
                        afvs[:, :],
                        grts[:, r * N:(r + 1) * N],
                        start=True,
                        stop=True,
                        tile_position=(0, base),
                    )
                nc.vector.tensor_copy(zt[:, :], ps_grv[:, :, :])

            # ---- stage 4: gav (the big stream) ----
            with tc.tile_pool(name="psGav", bufs=1, space="PSUM") as psGav:
                psg = psGav.tile([32, M2], F32)
                for dm in range(32):
                    ga_t = gap.tile([128, 2, M2], BF, tag="ga")
                    if dm < 31:
                        nc.sync.dma_start(
                            out=ga_t,
                            in_=gaT[dm * 256:(dm + 1) * 256, :].rearrange(
                                "(two p) m -> p two m", two=2
                            ),
                        )
                    else:
                        nc.sync.dma_start(
                            out=ga_t[:, 0, :], in_=gaT[7936:8064, :]
                        )
                        nc.sync.dma_start(
                            out=ga_t[0:64, 1, :], in_=gaT[8064:8128, :]
                        )
                    for half in range(2):
                        kt = dm * 2 + half
                        kw = 64 if kt == NKT - 1 else 128
                        for mc in range(4):
                            nc.tensor.matmul(
                                psg[:, mc * 512:(mc + 1) * 512],
                                fps[0:kw, kt * D:(kt + 1) * D],
                                ga_t[0:kw, half, mc * 512:(mc + 1) * 512],
                                start=(kt == 0),
                                stop=(kt == NKT - 1),
                            )
                nc.vector.tensor_copy(
                    ztg[:].rearrange("d (r n) -> d r n", n=N),
                    psg[:].rearrange("d (n r) -> d r n", r=Ra),
                )

            # ---- stage 5: embedding MLP ----
            with tc.tile_pool(name="psE", bufs=2, space="PSUM") as psE:
                # A1[n, h] accumulation: rank-1 eb1 + 16 grv k-tiles (ready
                # early) + 16 gav pieces (the only tail after the big stream)
                ps1 = psE.tile([128, H], F32, tag="a1")
                nc.tensor.matmul(ps1, ones1, eb1rs, start=True, stop=False)
                for kt in range(16):
                    nc.tensor.matmul(
                        ps1,
                        zt[:, kt * N:(kt + 1) * N],
                        ew1s[:, kt, :],
                        start=False,
                        stop=False,
                    )
                for rp in range(Ra):
                    nc.tensor.matmul(
                        ps1,
                        ztg[0:32, rp * N:(rp + 1) * N],
                        ew1gs[0:32, rp, :],
                        start=False,
                        stop=(rp == Ra - 1),
                    )
                nc.scalar.activation(a1, ps1, GELU, bias=0.0, scale=1.0)
                # transpose A1 -> A1^T via PE
                for ht in range(4):
                    tr = psE.tile([128, N], BF, tag="tr")
                    nc.tensor.transpose(tr, a1[:, ht * 128:(ht + 1) * 128], ident)
                    nc.vector.tensor_copy(a1t[:, ht, :], tr)
                for ec in range(2):
                    ps2 = psE.tile([128, N], F32, tag="aef")
                    for ht in range(4):
                        nc.tensor.matmul(
                            ps2,
                            ew2s[:, ht, ec * 128:(ec + 1) * 128],
                            a1t[:, ht, :],
                            start=(ht == 0),
                            stop=(ht == 3),
                        )
                    nc.scalar.activation(
                        aeft[:, ec, :], ps2, IDENT, bias=eb2s[:, ec:ec + 1], scale=1.0,
                    )

                nc.sync.dma_start(
                    out=out[:].rearrange("(c e) n -> e c n", c=2), in_=aeft
                )

    nc.compile()
    return nc


def _get_nc():
    if "nc" not in _CACHE:
        _CACHE["nc"] = _build_nc()
    return _CACHE["nc"]


def _prep_in_maps(gr, ga, afv, cw1, cb1, cw2, cb2, ew1, eb1, ew2, eb2):
    gr = np.asarray(gr, np.float32)
    ga = np.asarray(ga, np.float32)
    afv = np.asarray(afv, np.float32)
    cw1 = np.asarray(cw1, np.float32)
    cb1 = np.asarray(cb1, np.float32)
    cw2 = np.asarray(cw2, np.float32)
    cb2 = np.asarray(cb2, np.float32)
    ew1 = np.asarray(ew1, np.float32)
    eb1 = np.asarray(eb1, np.float32)
    ew2 = np.asarray(ew2, np.float32)
    eb2 = np.asarray(eb2, np.float32)

    shared = {
        "cw1": np.ascontiguousarray(cw1.astype(BF16NP)),
        "cw2": np.ascontiguousarray(cw2.astype(BF16NP)),
        "cb1": np.ascontiguousarray(cb1.reshape(2 * A, 1)),
        "cb2b": np.ascontiguousarray(np.broadcast_to(cb2, (2 * A, D))),
        "ew1": np.ascontiguousarray(ew1.astype(BF16NP)),
        "eb1r": np.ascontiguousarray(eb1.reshape(1, H).astype(BF16NP)),
        "ew2": np.ascontiguousarray(ew2.astype(BF16NP)),
        "eb2": np.ascontiguousarray(eb2),
    }
    in_maps = []
    ones64 = np.ones((A, N), np.float32)
    zeros64 = np.zeros((A, N), np.float32)
    for b in range(B):
        afvT = np.ascontiguousarray(afv[b].T)  # [64, 128]
        m = dict(shared)
        m["gaT"] = np.ascontiguousarray(
            ga[b].reshape(M2, P).T.astype(BF16NP)
        )
        # [m, (r, n)] r-major so per-r rhs slices are contiguous
        m["grT"] = np.ascontiguousarray(
            gr[b].transpose(2, 1, 0).reshape(N, G2).astype(BF16NP)
        )
        m["afv"] = np.ascontiguousarray(afv[b].astype(BF16NP))
        m["afv2"] = np.ascontiguousarray(np.concatenate([afvT, afvT], axis=0))
        m["s1"] = np.ascontiguousarray(np.concatenate([ones64, afvT], axis=0))
        m["s2"] = np.ascontiguousarray(np.concatenate([afvT, zeros64], axis=0))
        in_maps.append(m)
    return in_maps


def run(inputs: dict, trace: bool = False):
    """Returns ((aef, afv), exec_time_ns_or_None)."""
    nc = _get_nc()
    in_maps = _prep_in_maps(**inputs)
    res = run_bass_kernel_spmd(nc, in_maps, core_ids=list(range(B)), trace=trace)
    aef = np.stack(
        [np.ascontiguousarray(res.results[b]["out"].T) for b in range(B)], axis=0
    )
    afv = np.asarray(inputs["afv"], np.float32)
    return (aef, afv), res.exec_time_ns


def kernel(**inputs) -> np.ndarray:
    (aef, afv), _ = run(inputs, trace=False)
    return aef, afv
